# revision 66
# baseline (speedup 1.0000x reference)
"""GAT encoder (10-layer, JK-concat) Trainium2 Bass kernel — 8-core node-parallel.

v2 design (single collective per layer):
  - Linearity rewrite: out_l = W_l^T (sum_e alpha_e x~[s]) + b, logits via
    a~ = W_l @ a  =>  the gather table holds the post-gelu state x~_l, the
    dense matmul moves AFTER aggregation, and layer 0's table is just the raw
    input x (replicated; no collective).
  - Per layer: aggregate -> dense(+bias) -> pairnorm stats (local partials)
    -> ONE AllGather shipping pre-norm out rows + f32 stats packed as bf16
    hi/lo tail rows. Receivers reduce stats locally, then normalize+gelu the
    full table redundantly (cheap flat DVE/ACT ops) to produce the next
    gather table. This removes the second (stats) collective per layer that
    cost ~3.2ms each in this environment.
  - Edge phase: degree-sorted dst tiles, lo/hi int16 gather tables.
    Per-tile round ranges are processed with grouped instructions (this
    environment costs ~475ns per dynamic instruction, per engine queue):
    logit dots as one TT-multiply + one 3D-strided reduce per <=16 rounds;
    weight apply as one stride-0-broadcast TT (even tiles, DVE) or
    per-round ACT copies (odd tiles, engine balance); aggregation as
    stride-0-output matmuls accumulating 4 rounds per instruction into one
    PSUM tile.
"""

import numpy as np
import ml_dtypes
from contextlib import ExitStack

import concourse.bass as bass
import concourse.bacc as bacc
import concourse.tile as tile
import concourse.mybir as mybir

F32 = mybir.dt.float32
F32R = mybir.dt.float32r
BF16 = mybir.dt.bfloat16
I16 = mybir.dt.int16
AX = mybir.AxisListType
OP = mybir.AluOpType
AF = mybir.ActivationFunctionType

N = 50000
E = 640000
IN = 128
HID = 128
L = 10
NC = 8
NSH = N // NC          # 6250
TILES = 49
NSHP = TILES * 128     # 6272
NSHA = 6400            # 50*128: shard stride in the AG table (stats tail)
TBL = NSHA * NC        # 51200
PAIRS = TBL // 2       # 25600 512B pair-rows (< 32768: int16 covers all)
NEG = 0.2
PEPS = 1e-5
SEPS = 1e-16
SEG_MAX_ROUNDS = 40
GRP = 8                # rounds per grouped-dot/apply instruction
SINGLE_PACKET = False
N_SWDGE_Q = 4          # split each seg's gather across 4 SWDGE queues
NORM_CHUNK = 3200      # cols per table-normalization chunk (25 tiles)


def preprocess(edge_index):
    """Static graph preprocessing (pair-packed table: 512B rows hold 2 nodes,
    so 25600 pair-rows fit int16 with no lo/hi split). Returns (meta,
    percore): meta has round counts/segments; percore has the int16
    pair-index arrays + half-selection masks per core."""
    src = np.asarray(edge_index[0], dtype=np.int64)
    dst = np.asarray(edge_index[1], dtype=np.int64)
    owner = dst // NSH

    orders = []
    inv_all = np.empty(N, np.int64)   # global node -> sorted pos within owner
    for c in range(NC):
        m = owner == c
        dloc = dst[m] - c * NSH
        deg = np.bincount(dloc, minlength=NSH)
        order = np.argsort(-deg, kind="stable")
        inv = np.empty(NSH, np.int64)
        inv[order] = np.arange(NSH)
        orders.append(order)
        inv_all[c * NSH:(c + 1) * NSH] = inv
    tblrow_of_src = (src // NSH) * NSHA + inv_all[src]

    # per-core per-dst edge lists (global table rows)
    ed_lists = [[[] for _ in range(NSHP)] for _ in range(NC)]
    for c in range(NC):
        m = owner == c
        rows = tblrow_of_src[m]
        dpos = inv_all[dst[m]]
        o = np.argsort(dpos, kind="stable")
        rows = rows[o]
        dpos = dpos[o]
        counts = np.bincount(dpos, minlength=NSH)
        starts = np.concatenate([[0], np.cumsum(counts)])
        for p in range(NSH):
            ed_lists[c][p] = rows[starts[p]:starts[p + 1]]

    # common round structure (max over cores) — full degree, no split
    D = np.zeros(TILES, np.int64)
    for t in range(TILES):
        for c in range(NC):
            for sl in range(128):
                p = t * 128 + sl
                D[t] = max(D[t], len(ed_lists[c][p]))
    rounds_tot = int(D.sum())

    # segments: greedy group tiles
    segs = []
    cur = []
    cur_r = 0
    for t in range(TILES):
        rt = int(D[t])
        if cur and cur_r + rt > SEG_MAX_ROUNDS:
            segs.append(cur)
            cur, cur_r = [], 0
        cur.append(t)
        cur_r += rt
    if cur:
        segs.append(cur)

    def wrap_idx(flat):
        n = len(flat)
        assert n % 16 == 0
        w = np.asarray(flat, np.int16).reshape(-1, 16).T  # [16, n/16]
        return np.tile(w, (8, 1))                          # [128, n/16]

    percore = []
    for c in range(NC):
        idx_blocks = []
        mask = np.zeros((128, 2 * rounds_tot), np.float32)
        mcol = {}
        col = 0
        for t in range(TILES):
            mcol[t] = col
            col += int(D[t])
        for seg in segs:
            flat = []
            for t in seg:
                for k in range(int(D[t])):
                    for sl in range(128):
                        p = t * 128 + sl
                        lst = ed_lists[c][p]
                        if k < len(lst):
                            row = int(lst[k])
                            flat.append(row >> 1)
                            mask[sl, 2 * (mcol[t] + k) + (row & 1)] = 1.0
                        else:
                            flat.append(0)
            if flat:
                idx_blocks.append(wrap_idx(flat))
        idx_all = np.concatenate(idx_blocks, axis=1) if idx_blocks else np.zeros((128, 1), np.int16)
        percore.append({"idx": idx_all, "mask": mask, "order": orders[c]})

    meta = {"D": D, "segs": segs, "rounds_tot": rounds_tot,
            "dmax": int(D.max())}
    meta["pad_eff"] = 128 * rounds_tot * NC / E
    return meta, percore


def build(nc, meta, n_layers=L, sim_safe=False, variant="full"):
    """Emit the full Bass program under a TileContext."""
    D, segs = meta["D"], meta["segs"]
    rounds_tot = meta["rounds_tot"]
    DMX = meta["dmax"]
    GELU = AF.Sigmoid if sim_safe else AF.Gelu
    do_coll = variant != "nocoll"

    # ---- DRAM tensors
    xrows_in = nc.dram_tensor("xrows", [TBL, 128], BF16, kind="ExternalInput")
    xown_in = nc.dram_tensor("xown", [NSHP, 128], BF16, kind="ExternalInput")
    idx_in = nc.dram_tensor("idx", [128, 8 * rounds_tot], I16, kind="ExternalInput")
    mask_in = nc.dram_tensor("mask", [128, 2 * rounds_tot], BF16, kind="ExternalInput")
    W_in = nc.dram_tensor("Wst", [n_layers, 128, 128], F32R, kind="ExternalInput")
    asrc_in = nc.dram_tensor("asrc", [n_layers, 128, GRP * 256], BF16, kind="ExternalInput")
    adst_in = nc.dram_tensor("adst", [n_layers, 128, GRP * 128], BF16, kind="ExternalInput")
    biasc_in = nc.dram_tensor("biasc", [128, n_layers], F32, kind="ExternalInput")
    linw_in = nc.dram_tensor("linw", [n_layers, 128, 128], F32R, kind="ExternalInput")
    linbr_in = nc.dram_tensor("linbr", [128, 128], F32, kind="ExternalInput")
    identb_in = nc.dram_tensor("identb", [128, 128], BF16, kind="ExternalInput")
    identf_in = nc.dram_tensor("identf", [128, 128], F32, kind="ExternalInput")
    ones_in = nc.dram_tensor("ones", [128, 128], F32, kind="ExternalInput")
    padm_in = nc.dram_tensor("padm", [128, 1], F32, kind="ExternalInput")
    y_out = nc.dram_tensor("y", [NSHP, 128], F32, kind="ExternalOutput")

    ag_in = nc.dram_tensor("ag_in", [NSHA, 128], BF16)
    table2 = nc.dram_tensor("table2", [TBL, 128], BF16, addr_space="Shared")
    table = nc.dram_tensor("table", [TBL, 128], BF16)
    st_in = nc.dram_tensor("st_in", [128, 2], F32)
    st_out = nc.dram_tensor("st_out", [NC * 128, 2], F32, addr_space="Shared")

    RG = [list(range(NC))]

    with tile.TileContext(nc) as tc, ExitStack() as ctx:
        # ---- persistent SBUF (bufs=1 pools)
        P = ctx.enter_context(tc.tile_pool(name="persist", bufs=1))
        rtab = P.tile([128, NSHP], BF16, tag="rtab")      # own x~_l rows
        outrows = P.tile([128, NSHP], BF16, tag="outrows")  # pre-norm out rows
        aggT = P.tile([128, NSHP], F32R, tag="aggT")      # agg^T / out^T slab
        outfin = P.tile([128, NSHP], F32, tag="outfin")
        idx_sb = P.tile([128, 8 * rounds_tot], I16, tag="idx")
        mask_sb = P.tile([128, 2 * rounds_tot], BF16, tag="mask")
        W_sb = P.tile([128, n_layers * 128], F32R, tag="W")
        asl_t = P.tile([128, GRP * 256], BF16, tag="asl_t")
        adl_t = P.tile([128, GRP * 128], BF16, tag="adl_t")
        biasc_sb = P.tile([128, n_layers], F32, tag="biasc")
        linw_sb = P.tile([128, n_layers * 128], F32R, tag="linw")
        linbr_sb = P.tile([128, 128], F32, tag="linbr")
        identb = P.tile([128, 128], BF16, tag="identb")
        identf = P.tile([128, 128], F32, tag="identf")
        ones_sb = P.tile([128, 128], F32, tag="ones")
        padm_sb = P.tile([128, 1], F32, tag="padm")
        as_own = P.tile([128, TILES], F32, tag="as_own")
        ad_own = P.tile([128, TILES], F32, tag="ad_own")
        wself = P.tile([128, TILES], F32, tag="wself")
        stp = P.tile([128, 2], F32, tag="stp")
        statp = P.tile([128, 4], BF16, tag="statp")
        strow = P.tile([4, 128], BF16, tag="strow")
        st32 = P.tile([32, 128], BF16, tag="st32")
        stf = P.tile([32, 128], F32, tag="stf")
        gst = P.tile([128, 32], F32, tag="gst")
        st8 = P.tile([128, 16], F32, tag="st8")
        gstat = P.tile([128, 2], F32, tag="gstat")
        nmslab = P.tile([128, NORM_CHUNK], F32, tag="nmslab")
        gtmp = P.tile([128, 16], F32, tag="gtmp")

        # ---- input loads
        nc.sync.dma_start(idx_sb[:], idx_in.ap())
        nc.sync.dma_start(mask_sb[:], mask_in.ap())
        nc.sync.dma_start(W_sb[:].rearrange("a (l b) -> a l b", b=128), W_in.ap().rearrange("l a b -> a l b"))
        nc.sync.dma_start(biasc_sb[:], biasc_in.ap())
        nc.sync.dma_start(linw_sb[:].rearrange("a (l b) -> a l b", b=128), linw_in.ap().rearrange("l a b -> a l b"))
        nc.sync.dma_start(linbr_sb[:], linbr_in.ap())
        nc.sync.dma_start(identb[:], identb_in.ap())
        nc.sync.dma_start(identf[:], identf_in.ap())
        nc.sync.dma_start(ones_sb[:], ones_in.ap())
        nc.sync.dma_start(padm_sb[:], padm_in.ap())
        nc.sync.dma_start(rtab[:].rearrange("p (t f) -> p t f", f=128),
                          xown_in.ap().rearrange("(t p) f -> p t f", p=128))

        # ---- working pools
        PD = ctx.enter_context(tc.tile_pool(name="pd", bufs=2, space="PSUM"))
        PT = ctx.enter_context(tc.tile_pool(name="pt", bufs=2, space="PSUM"))
        PA = ctx.enter_context(tc.tile_pool(name="pa", bufs=2, space="PSUM"))
        PM = ctx.enter_context(tc.tile_pool(name="pm", bufs=2, space="PSUM"))
        GSEG = ctx.enter_context(tc.tile_pool(name="gseg", bufs=2))
        GN = ctx.enter_context(tc.tile_pool(name="gn", bufs=2))
        JNK8 = ctx.enter_context(tc.tile_pool(name="jnk8", bufs=2))
        GW8 = ctx.enter_context(tc.tile_pool(name="gw8", bufs=2))
        SC = ctx.enter_context(tc.tile_pool(name="sc", bufs=3))
        SS = ctx.enter_context(tc.tile_pool(name="ss", bufs=4))
        RP = ctx.enter_context(tc.tile_pool(name="rp", bufs=3))

        # mask col offset per tile (processing order)
        mcol = {}
        col = 0
        for t in range(TILES):
            mcol[t] = col
            col += int(D[t])
        # idx col offsets (wrapped layout: 8 cols per round)
        idx_off = {}
        ic = 0
        for si, seg in enumerate(segs):
            seg_r = int(sum(D[t] for t in seg))
            idx_off[si] = (ic, seg_r)
            ic += 8 * seg_r

        for l in range(n_layers):
            src_tab = xrows_in if l == 0 else table
            wsl = W_sb[:, l * 128:(l + 1) * 128]
            lwl = linw_sb[:, l * 128:(l + 1) * 128]

            # ---- per-layer tiled attention vectors
            nc.sync.dma_start(asl_t[:], asrc_in.ap()[l])
            nc.sync.dma_start(adl_t[:], adst_in.ap()[l])

            # ---- own alpha dots (grouped: TT mult + 3D reduce) + self weight
            for t0 in range(0, TILES, GRP):
                g = min(GRP, TILES - t0)
                gsl = slice(t0 * 128, (t0 + g) * 128)
                for avec, aout in ((asl_t, as_own), (adl_t, ad_own)):
                    pr = JNK8.tile([128, GRP * 128], BF16, tag="jnk8")
                    nc.vector.tensor_tensor(pr[:, :g * 128], rtab[:, gsl],
                                            avec[:, :g * 128], op=OP.mult)
                    nc.vector.reduce_sum(
                        out=aout[:, t0:t0 + g],
                        in_=pr[:, :g * 128].rearrange("p (r e) -> p r e", e=128),
                        axis=AX.X)
            zs = SC.tile([128, TILES], F32, tag="zself")
            nc.vector.tensor_tensor(zs[:], as_own[:], ad_own[:], op=OP.add)
            zs2 = SC.tile([128, TILES], F32, tag="zself")
            nc.vector.scalar_tensor_tensor(
                out=zs2[:], in0=zs[:], scalar=NEG, in1=zs[:], op0=OP.mult, op1=OP.max)
            nc.scalar.activation(wself[:], zs2[:], AF.Exp)

            # ---- aggregation over segments (pair-packed 512B gathers)
            src_pairs = src_tab.ap().rearrange("(q two) f -> q (two f)", two=2)
            for si, seg in enumerate(segs):
                seg_r = int(sum(D[t] for t in seg))
                gbuf = GSEG.tile([128, SEG_MAX_ROUNDS * 256], BF16, tag="gseg")
                g3 = gbuf[:].rearrange("p (r e) -> p r e", e=256)
                ic0, _sr = idx_off[si]
                if variant == "nogather":
                    nc.vector.memset(gbuf[:, :seg_r * 256], 0.0)
                else:
                    bnds = sorted({seg_r * j // N_SWDGE_Q
                                   for j in range(N_SWDGE_Q + 1)})
                    for qi in range(len(bnds) - 1):
                        b0, b1 = bnds[qi], bnds[qi + 1]
                        nc.gpsimd.dma_gather(
                            g3[:, b0:b1, :], src_pairs,
                            idx_sb[:, ic0 + 8 * b0:ic0 + 8 * b1],
                            128 * (b1 - b0), 128 * (b1 - b0), 256,
                            single_packet=SINGLE_PACKET, queue_num=qi)

                # per-tile processing (2 interleaved halves per round)
                cum = 0
                for t in seg:
                    Dt = int(D[t])
                    tsl = slice(t * 128, (t + 1) * 128)
                    c0 = cum
                    asg = SC.tile([128, 2 * DMX], F32, tag="asg")
                    for q0 in range(0, Dt, GRP):
                        q = min(GRP, Dt - q0)
                        pr = JNK8.tile([128, GRP * 256], BF16, tag="jnk8")
                        nc.vector.tensor_tensor(
                            pr[:, :q * 256],
                            gbuf[:, (c0 + q0) * 256:(c0 + q0 + q) * 256],
                            asl_t[:, :q * 256], op=OP.mult)
                        nc.vector.reduce_sum(
                            out=asg[:, 2 * q0:2 * (q0 + q)],
                            in_=pr[:, :q * 256].rearrange(
                                "p (r e) -> p r e", e=128),
                            axis=AX.X)
                    zt = SC.tile([128, 2 * DMX], F32, tag="zt")
                    nc.vector.tensor_scalar_add(zt[:, :2 * Dt], asg[:, :2 * Dt],
                                                ad_own[:, t:t + 1])
                    zl = SC.tile([128, 2 * DMX], F32, tag="zl")
                    nc.vector.scalar_tensor_tensor(
                        out=zl[:, :2 * Dt], in0=zt[:, :2 * Dt], scalar=NEG,
                        in1=zt[:, :2 * Dt], op0=OP.mult, op1=OP.max)
                    ew = SC.tile([128, 2 * DMX], F32, tag="ew")
                    nc.scalar.activation(ew[:, :2 * Dt], zl[:, :2 * Dt], AF.Exp)
                    ewm = SC.tile([128, 2 * DMX], F32, tag="ewm")
                    S = SS.tile([128, 1], F32, tag="S")
                    nc.vector.scalar_tensor_tensor(
                        out=ewm[:, :2 * Dt], in0=ew[:, :2 * Dt], scalar=1.0,
                        in1=mask_sb[:, 2 * mcol[t]:2 * (mcol[t] + Dt)],
                        op0=OP.mult, op1=OP.mult, accum_out=S[:])
                    Sp = SS.tile([128, 1], F32, tag="Sp")
                    nc.vector.scalar_tensor_tensor(
                        out=Sp[:], in0=S[:], scalar=SEPS, in1=wself[:, t:t + 1],
                        op0=OP.add, op1=OP.add)
                    rec = SS.tile([128, 1], F32, tag="rec")
                    nc.vector.reciprocal(rec[:], Sp[:])

                    pa = PA.tile([128, 128], F32, tag="pa")
                    pav = pa[:]
                    nmm = -(-(2 * Dt) // 4)
                    mi = 0
                    use_dve = (t % 2 == 0)
                    for q0 in range(0, Dt, GRP):
                        q = min(GRP, Dt - q0)
                        gw = GW8.tile([128, GRP * 256], BF16, tag="gw8")
                        if use_dve:
                            ew_ap = ewm[:, 2 * q0:2 * (q0 + q)]
                            e3 = bass.AP(
                                ew_ap.tensor, ew_ap.offset,
                                [list(ew_ap.ap[0]),
                                 [ew_ap.ap[1][0], 2 * q], [0, 128]])
                            nc.vector.tensor_tensor(
                                gw[:, :q * 256].rearrange(
                                    "p (r e) -> p r e", e=128),
                                gbuf[:, (c0 + q0) * 256:(c0 + q0 + q) * 256]
                                .rearrange("p (r e) -> p r e", e=128),
                                e3, op=OP.mult)
                        else:
                            for kk in range(2 * q):
                                nc.scalar.activation(
                                    gw[:, kk * 128:(kk + 1) * 128],
                                    gbuf[:, (c0 + q0) * 256 + kk * 128:
                                         (c0 + q0) * 256 + (kk + 1) * 128],
                                    AF.Copy,
                                    scale=ewm[:, 2 * q0 + kk:2 * q0 + kk + 1])
                        for m0 in range(0, 2 * q, 4):
                            m = min(4, 2 * q - m0)
                            pz = bass.AP(
                                pav.tensor, pav.offset,
                                [list(pav.ap[0]), [0, m], list(pav.ap[1])])
                            nc.tensor.matmul(
                                pz, identb[:],
                                gw[:, m0 * 128:(m0 + m) * 128],
                                start=(mi == 0), stop=(mi == nmm - 1))
                            mi += 1

                    acc1 = RP.tile([128, 128], F32, tag="acc1")
                    nc.vector.scalar_tensor_tensor(
                        out=acc1[:], in0=rtab[:, tsl], scalar=wself[:, t:t + 1],
                        in1=pa[:], op0=OP.mult, op1=OP.add)
                    row = RP.tile([128, 128], F32, tag="row")
                    nc.scalar.activation(row[:], acc1[:], AF.Copy, scale=rec[:])
                    if t == TILES - 1:
                        nc.vector.tensor_scalar_mul(row[:], row[:], padm_sb[:])
                    # transpose agg rows into aggT slab
                    ptf = PM.tile([128, 128], F32, tag="pm")
                    nc.tensor.transpose(ptf[:], row[:], identf[:])
                    nc.vector.tensor_copy(aggT[:, tsl], ptf[:])
                    cum += Dt

            # ---- dense post-aggregation: out^T = W^T @ agg^T + bias
            for ch0 in range(0, NSHP, 512):
                chsz = min(512, NSHP - ch0)
                pd = PD.tile([128, 512], F32, tag="pd")
                nc.tensor.matmul(pd[:, :chsz], wsl, aggT[:, ch0:ch0 + chsz],
                                 start=True, stop=True)
                nc.vector.tensor_scalar_add(aggT[:, ch0:ch0 + chsz], pd[:, :chsz],
                                            biasc_sb[:, l:l + 1])

            # ---- pairnorm stats partials on out^T (feat-major)
            fsum = SS.tile([128, 1], F32, tag="fsum")
            nc.vector.reduce_sum(out=fsum[:], in_=aggT[:], axis=AX.X)
            sqc = SS.tile([128, 1], F32, tag="sqc")
            nc.scalar.activation(outrows[:], aggT[:], AF.Square, accum_out=sqc[:])
            nc.vector.tensor_copy(stp[:, :1], fsum[:])
            nc.vector.tensor_copy(stp[:, 1:2], sqc[:])

            if l < n_layers - 1:
                # ---- payload: transpose out^T -> out rows (bf16)
                for t in range(TILES):
                    tsl = slice(t * 128, (t + 1) * 128)
                    ptf = PM.tile([128, 128], F32, tag="pm")
                    nc.tensor.transpose(ptf[:], aggT[:, tsl].bitcast(F32),
                                        identf[:])
                    nc.scalar.activation(outrows[:, tsl], ptf[:], AF.Copy)
                # stats hi/lo pack into statp [128,4] = (fh, qh, fl, ql) cols
                nc.vector.tensor_copy(statp[:, 0:2], stp[:])
                hif = SS.tile([128, 2], F32, tag="hif")
                nc.vector.tensor_copy(hif[:], statp[:, 0:2])
                lof = SS.tile([128, 2], F32, tag="lof")
                nc.vector.tensor_tensor(lof[:], stp[:], hif[:], op=OP.subtract)
                nc.vector.tensor_copy(statp[:, 2:4], lof[:])
                pst = PT.tile([128, 128], BF16, tag="pt")
                nc.tensor.transpose(pst[:4, :], statp[:], identb[:])
                nc.vector.tensor_copy(strow[:], pst[:4, :])
                # DMA payload
                nc.sync.dma_start(ag_in.ap()[:NSHP].rearrange("(t p) f -> p t f", p=128),
                                  outrows[:].rearrange("p (t f) -> p t f", f=128))
                nc.sync.dma_start(ag_in.ap()[NSHP:NSHP + 4], strow[:])
                if do_coll:
                    nc.gpsimd.collective_compute(
                        "AllGather", OP.bypass, replica_groups=RG,
                        ins=[ag_in.ap()], outs=[table2.ap()])
                else:
                    nc.sync.dma_start(table2.ap()[:NSHA], ag_in.ap())

                # ---- global stats from the 8 cores' tail rows
                for cc in range(NC):
                    nc.sync.dma_start(
                        st32[cc * 4:(cc + 1) * 4, :],
                        table2.ap()[cc * NSHA + NSHP:cc * NSHA + NSHP + 4, :])
                nc.vector.tensor_copy(stf[:], st32[:])
                pg = PM.tile([128, 128], F32, tag="pm")
                nc.tensor.transpose(pg[:, :32], stf[:], identf[:32, :32])
                nc.vector.tensor_copy(gst[:], pg[:, :32])
                g4 = gst[:].rearrange("p (c a) -> p c a", a=4)
                nc.vector.tensor_tensor(
                    st8[:].rearrange("p (c s) -> p c s", s=2),
                    g4[:, :, 0:2], g4[:, :, 2:4], op=OP.add)
                nc.vector.reduce_sum(
                    out=gstat[:],
                    in_=st8[:].rearrange("p (c s) -> p s c", s=2),
                    axis=AX.X)
            else:
                # ---- last layer: lone stats AllGather
                nc.sync.dma_start(st_in.ap(), stp[:])
                if do_coll:
                    nc.gpsimd.collective_compute(
                        "AllGather", OP.bypass, replica_groups=RG,
                        ins=[st_in.ap()], outs=[st_out.ap()])
                else:
                    for rr in range(NC):
                        nc.sync.dma_start(st_out.ap()[rr * 128:(rr + 1) * 128],
                                          st_in.ap())
                nc.sync.dma_start(gtmp[:].rearrange("p (r c) -> p r c", c=2),
                                  st_out.ap().rearrange("(r p) c -> p r c", p=128))
                nc.vector.reduce_sum(out=gstat[:],
                                     in_=gtmp[:].rearrange("p (r c) -> p c r", c=2),
                                     axis=AX.X)

            # ---- mu / denom / scale (gstat = [fsum_g, sqsum_g] per feature)
            mu = SS.tile([128, 1], F32, tag="mu")
            nc.vector.tensor_scalar_mul(mu[:], gstat[:, :1], 1.0 / N)
            st2 = SS.tile([128, 2], F32, tag="st2")
            nc.vector.tensor_copy(st2[:, :1], gstat[:, 1:2])
            nc.vector.tensor_tensor(st2[:, 1:2], mu[:], mu[:], op=OP.mult)
            p2 = PM.tile([128, 128], F32, tag="pm")
            nc.tensor.matmul(p2[:1, :2], ones_sb[:, :1], st2[:],
                             start=True, stop=True)
            tot = SS.tile([1, 2], F32, tag="tot")
            nc.vector.tensor_copy(tot[:], p2[:1, :2])
            v3 = SS.tile([1, 1], F32, tag="v3")
            nc.vector.tensor_scalar(v3[:], tot[:, :1], 1.0 / N, PEPS,
                                    op0=OP.mult, op1=OP.add)
            v4 = SS.tile([1, 1], F32, tag="v4")
            nc.vector.tensor_tensor(v4[:], v3[:], tot[:, 1:2], op=OP.subtract)
            den = SS.tile([1, 1], F32, tag="den")
            nc.scalar.activation(den[:], v4[:], AF.Sqrt)
            invd = SS.tile([1, 1], F32, tag="invd")
            nc.vector.reciprocal(invd[:], den[:])
            pb1 = PM.tile([128, 128], F32, tag="pm")
            nc.tensor.matmul(pb1[:, :1], ones_sb[:1, :], invd[:],
                             start=True, stop=True)
            invdr = SS.tile([128, 1], F32, tag="invdr")
            nc.vector.tensor_copy(invdr[:], pb1[:, :1])
            nms = SS.tile([128, 1], F32, tag="nms")
            nc.vector.tensor_scalar(nms[:], mu[:], invdr[:], -1.0,
                                    op0=OP.mult, op1=OP.mult)

            # ---- own feat-major norm+gelu (in place on aggT) + JK increment
            for ch0 in range(0, NSHP, 512):
                chsz = min(512, NSHP - ch0)
                csl = slice(ch0, ch0 + chsz)
                nc.scalar.activation(aggT[:, csl], aggT[:, csl], GELU,
                                     bias=nms[:], scale=invdr[:])
            for t in range(TILES):
                tsl = slice(t * 128, (t + 1) * 128)
                pf = PM.tile([128, 128], F32, tag="pm")
                nc.tensor.matmul(pf[:], aggT[:, tsl], lwl, start=True, stop=True)
                if l == 0:
                    nc.vector.scalar_tensor_tensor(
                        out=outfin[:, tsl], in0=pf[:], scalar=1.0, in1=linbr_sb[:],
                        op0=OP.mult, op1=OP.add)
                else:
                    nc.vector.scalar_tensor_tensor(
                        out=outfin[:, tsl], in0=pf[:], scalar=1.0,
                        in1=outfin[:, tsl], op0=OP.mult, op1=OP.add)

            if l < n_layers - 1:
                # ---- nms replicated-row slab for row-space norm
                pnr = PM.tile([128, 128], F32, tag="pm")
                nc.tensor.transpose(pnr[:1, :], nms[:], identf[:])
                nmsrow = SS.tile([1, 128], F32, tag="nmsrow")
                nc.vector.tensor_copy(nmsrow[:], pnr[:1, :])
                prep = PM.tile([128, 128], F32, tag="pm")
                nc.tensor.matmul(prep[:], ones_sb[:1, :], nmsrow[:],
                                 start=True, stop=True)
                nrep = SS.tile([128, 128], F32, tag="nrep")
                nc.vector.tensor_copy(nrep[:], prep[:])
                for k in range(NORM_CHUNK // 128):
                    nc.vector.tensor_copy(nmslab[:, k * 128:(k + 1) * 128], nrep[:])

                # ---- own rows norm+gelu -> rtab (next layer's own rows)
                for h0, hw in ((0, 3072), (3072, NSHP - 3072)):
                    hs = slice(h0, h0 + hw)
                    nc.vector.scalar_tensor_tensor(
                        out=outrows[:, hs], in0=outrows[:, hs], scalar=invdr[:],
                        in1=nmslab[:, :hw], op0=OP.mult, op1=OP.add)
                    nc.scalar.activation(rtab[:, hs], outrows[:, hs], GELU)

                # ---- full-table norm+gelu: table2 -> table (chunked)
                nch = TBL // NORM_CHUNK
                for ci in range(nch):
                    r0 = ci * NORM_CHUNK
                    cin = GN.tile([128, NORM_CHUNK], BF16, tag="gnb")
                    nc.sync.dma_start(
                        cin[:].rearrange("p (t f) -> p t f", f=128),
                        table2.ap()[r0:r0 + NORM_CHUNK].rearrange(
                            "(t p) f -> p t f", p=128))
                    nc.vector.scalar_tensor_tensor(
                        out=cin[:], in0=cin[:], scalar=invdr[:],
                        in1=nmslab[:], op0=OP.mult, op1=OP.add)
                    nc.scalar.activation(cin[:], cin[:], GELU)
                    nc.sync.dma_start(
                        table.ap()[r0:r0 + NORM_CHUNK].rearrange(
                            "(t p) f -> p t f", p=128),
                        cin[:].rearrange("p (t f) -> p t f", f=128))

        # ---- write final output
        nc.sync.dma_start(y_out.ap().rearrange("(t p) f -> p t f", p=128),
                          outfin[:].rearrange("p (t f) -> p t f", f=128))

    return nc


def make_inputs(inputs, meta, percore, n_layers=L):
    """Build per-core in_maps from the full problem inputs."""
    x = np.asarray(inputs["x"], np.float32)
    W0 = np.asarray(inputs["W0"], np.float32)
    Ws = np.asarray(inputs["Ws"], np.float32)
    att_src = np.asarray(inputs["att_src"], np.float32)
    att_dst = np.asarray(inputs["att_dst"], np.float32)
    bias = np.asarray(inputs["bias"], np.float32)
    lin_w = np.asarray(inputs["lin_w"], np.float32)
    lin_b = np.asarray(inputs["lin_b"], np.float32)

    Wl = [W0] + [Ws[i] for i in range(n_layers - 1)]
    Wst = np.stack(Wl).astype(np.float32)
    asrc = np.stack([np.tile(Wl[i] @ att_src[i], (128, 2 * GRP))
                     for i in range(n_layers)]).astype(ml_dtypes.bfloat16)
    adst = np.stack([np.tile(Wl[i] @ att_dst[i], (128, GRP))
                     for i in range(n_layers)]).astype(ml_dtypes.bfloat16)
    biasc = np.stack([bias[i] for i in range(n_layers)], axis=1).astype(np.float32)
    linw = np.stack([lin_w[i * HID:(i + 1) * HID] for i in range(n_layers)]).astype(np.float32)
    linbr = np.tile(lin_b, (128, 1)).astype(np.float32)
    identb = np.eye(128, dtype=ml_dtypes.bfloat16)
    identf = np.eye(128, dtype=np.float32)
    ones = np.ones((128, 128), np.float32)
    padm = np.zeros((128, 1), np.float32)
    padm[:NSH - (TILES - 1) * 128] = 1.0

    # full layer-0 gather table: x rows in permuted layout, replicated
    xrows = np.zeros((TBL, 128), ml_dtypes.bfloat16)
    xb = x.astype(ml_dtypes.bfloat16)
    in_maps = []
    xowns = []
    for c in range(NC):
        order = percore[c]["order"]
        xs = xb[c * NSH:(c + 1) * NSH][order]          # [6250,128] sorted
        xrows[c * NSHA:c * NSHA + NSH] = xs
        xo = np.zeros((NSHP, 128), ml_dtypes.bfloat16)
        xo[:NSH] = xs
        xowns.append(xo)
    for c in range(NC):
        in_maps.append({
            "xrows": xrows, "xown": xowns[c],
            "idx": percore[c]["idx"],
            "mask": percore[c]["mask"].astype(ml_dtypes.bfloat16),
            "Wst": Wst, "asrc": asrc, "adst": adst, "biasc": biasc,
            "linw": linw, "linbr": linbr, "identb": identb, "identf": identf,
            "ones": ones, "padm": padm,
        })
    return in_maps


def assemble_output(results, percore):
    """Concatenate per-core outputs, undoing the degree-sort permutation."""
    out = np.empty((N, HID), np.float32)
    for c in range(NC):
        order = percore[c]["order"]
        yc = results[c]["y"][:NSH]     # sorted order
        out[c * NSH + order] = yc
    return out


# ---------------------------------------------------------------------------
# kernel() entry point
# ---------------------------------------------------------------------------
_CACHE = {}


def _get_compiled(edge_key, edge_index):
    if edge_key not in _CACHE:
        meta, percore = preprocess(edge_index)
        nc = bacc.Bacc("TRN2", target_bir_lowering=False, debug=False,
                       num_devices=NC, num_swdge_queues=N_SWDGE_Q)
        build(nc, meta, n_layers=L, sim_safe=False)
        nc.compile()
        _CACHE[edge_key] = (nc, meta, percore)
    return _CACHE[edge_key]


def kernel(**inputs):
    from concourse.bass_utils import run_bass_kernel_spmd
    edge_index = np.asarray(inputs["edge_index"])
    edge_key = hash(edge_index.tobytes())
    nc, meta, percore = _get_compiled(edge_key, edge_index)
    in_maps = make_inputs(inputs, meta, percore, n_layers=L)
    res = run_bass_kernel_spmd(nc, in_maps, list(range(NC)))
    return assemble_output(res.results, percore)


# revision 67
# speedup vs baseline: 1.0120x; 1.0120x over previous
"""GAT encoder (10-layer, JK-concat) Trainium2 Bass kernel — 8-core node-parallel.

v2 design (single collective per layer):
  - Linearity rewrite: out_l = W_l^T (sum_e alpha_e x~[s]) + b, logits via
    a~ = W_l @ a  =>  the gather table holds the post-gelu state x~_l, the
    dense matmul moves AFTER aggregation, and layer 0's table is just the raw
    input x (replicated; no collective).
  - Per layer: aggregate -> dense(+bias) -> pairnorm stats (local partials)
    -> ONE AllGather shipping pre-norm out rows + f32 stats packed as bf16
    hi/lo tail rows. Receivers reduce stats locally, then normalize+gelu the
    full table redundantly (cheap flat DVE/ACT ops) to produce the next
    gather table. This removes the second (stats) collective per layer that
    cost ~3.2ms each in this environment.
  - Edge phase: degree-sorted dst tiles, lo/hi int16 gather tables.
    Per-tile round ranges are processed with grouped instructions (this
    environment costs ~475ns per dynamic instruction, per engine queue):
    logit dots as one TT-multiply + one 3D-strided reduce per <=16 rounds;
    weight apply as one stride-0-broadcast TT (even tiles, DVE) or
    per-round ACT copies (odd tiles, engine balance); aggregation as
    stride-0-output matmuls accumulating 4 rounds per instruction into one
    PSUM tile.
"""

import numpy as np
import ml_dtypes
from contextlib import ExitStack

import concourse.bass as bass
import concourse.bacc as bacc
import concourse.tile as tile
import concourse.mybir as mybir

F32 = mybir.dt.float32
F32R = mybir.dt.float32r
BF16 = mybir.dt.bfloat16
I16 = mybir.dt.int16
AX = mybir.AxisListType
OP = mybir.AluOpType
AF = mybir.ActivationFunctionType

N = 50000
E = 640000
IN = 128
HID = 128
L = 10
NC = 8
NSH = N // NC          # 6250
TILES = 49
NSHP = TILES * 128     # 6272
NSHA = 6400            # 50*128: shard stride in the AG table (stats tail)
TBL = NSHA * NC        # 51200
PAIRS = TBL // 2       # 25600 512B pair-rows (< 32768: int16 covers all)
NEG = 0.2
PEPS = 1e-5
SEPS = 1e-16
SEG_MAX_ROUNDS = 40
GRP = 8                # rounds per grouped-dot/apply instruction
SINGLE_PACKET = False
N_SWDGE_Q = 2          # split each seg's gather across 2 SWDGE queues
NORM_CHUNK = 3200      # cols per table-normalization chunk (25 tiles)


def preprocess(edge_index):
    """Static graph preprocessing (pair-packed table: 512B rows hold 2 nodes,
    so 25600 pair-rows fit int16 with no lo/hi split). Returns (meta,
    percore): meta has round counts/segments; percore has the int16
    pair-index arrays + half-selection masks per core."""
    src = np.asarray(edge_index[0], dtype=np.int64)
    dst = np.asarray(edge_index[1], dtype=np.int64)
    owner = dst // NSH

    orders = []
    inv_all = np.empty(N, np.int64)   # global node -> sorted pos within owner
    for c in range(NC):
        m = owner == c
        dloc = dst[m] - c * NSH
        deg = np.bincount(dloc, minlength=NSH)
        order = np.argsort(-deg, kind="stable")
        inv = np.empty(NSH, np.int64)
        inv[order] = np.arange(NSH)
        orders.append(order)
        inv_all[c * NSH:(c + 1) * NSH] = inv
    tblrow_of_src = (src // NSH) * NSHA + inv_all[src]

    # per-core per-dst edge lists (global table rows)
    ed_lists = [[[] for _ in range(NSHP)] for _ in range(NC)]
    for c in range(NC):
        m = owner == c
        rows = tblrow_of_src[m]
        dpos = inv_all[dst[m]]
        o = np.argsort(dpos, kind="stable")
        rows = rows[o]
        dpos = dpos[o]
        counts = np.bincount(dpos, minlength=NSH)
        starts = np.concatenate([[0], np.cumsum(counts)])
        for p in range(NSH):
            ed_lists[c][p] = rows[starts[p]:starts[p + 1]]

    # common round structure (max over cores) — full degree, no split
    D = np.zeros(TILES, np.int64)
    for t in range(TILES):
        for c in range(NC):
            for sl in range(128):
                p = t * 128 + sl
                D[t] = max(D[t], len(ed_lists[c][p]))
    rounds_tot = int(D.sum())

    # segments: greedy group tiles
    segs = []
    cur = []
    cur_r = 0
    for t in range(TILES):
        rt = int(D[t])
        if cur and cur_r + rt > SEG_MAX_ROUNDS:
            segs.append(cur)
            cur, cur_r = [], 0
        cur.append(t)
        cur_r += rt
    if cur:
        segs.append(cur)

    def wrap_idx(flat):
        n = len(flat)
        assert n % 16 == 0
        w = np.asarray(flat, np.int16).reshape(-1, 16).T  # [16, n/16]
        return np.tile(w, (8, 1))                          # [128, n/16]

    percore = []
    for c in range(NC):
        idx_blocks = []
        mask = np.zeros((128, 2 * rounds_tot), np.float32)
        mcol = {}
        col = 0
        for t in range(TILES):
            mcol[t] = col
            col += int(D[t])
        for seg in segs:
            flat = []
            for t in seg:
                for k in range(int(D[t])):
                    for sl in range(128):
                        p = t * 128 + sl
                        lst = ed_lists[c][p]
                        if k < len(lst):
                            row = int(lst[k])
                            flat.append(row >> 1)
                            mask[sl, 2 * (mcol[t] + k) + (row & 1)] = 1.0
                        else:
                            flat.append(0)
            if flat:
                idx_blocks.append(wrap_idx(flat))
        idx_all = np.concatenate(idx_blocks, axis=1) if idx_blocks else np.zeros((128, 1), np.int16)
        percore.append({"idx": idx_all, "mask": mask, "order": orders[c]})

    meta = {"D": D, "segs": segs, "rounds_tot": rounds_tot,
            "dmax": int(D.max())}
    meta["pad_eff"] = 128 * rounds_tot * NC / E
    return meta, percore


def build(nc, meta, n_layers=L, sim_safe=False, variant="full"):
    """Emit the full Bass program under a TileContext."""
    D, segs = meta["D"], meta["segs"]
    rounds_tot = meta["rounds_tot"]
    DMX = meta["dmax"]
    GELU = AF.Sigmoid if sim_safe else AF.Gelu
    do_coll = variant != "nocoll"

    # ---- DRAM tensors
    xrows_in = nc.dram_tensor("xrows", [TBL, 128], BF16, kind="ExternalInput")
    xown_in = nc.dram_tensor("xown", [NSHP, 128], BF16, kind="ExternalInput")
    idx_in = nc.dram_tensor("idx", [128, 8 * rounds_tot], I16, kind="ExternalInput")
    mask_in = nc.dram_tensor("mask", [128, 2 * rounds_tot], BF16, kind="ExternalInput")
    W_in = nc.dram_tensor("Wst", [n_layers, 128, 128], F32R, kind="ExternalInput")
    asrc_in = nc.dram_tensor("asrc", [n_layers, 128, GRP * 256], BF16, kind="ExternalInput")
    adst_in = nc.dram_tensor("adst", [n_layers, 128, GRP * 128], BF16, kind="ExternalInput")
    biasc_in = nc.dram_tensor("biasc", [128, n_layers], F32, kind="ExternalInput")
    linw_in = nc.dram_tensor("linw", [n_layers, 128, 128], F32R, kind="ExternalInput")
    linbr_in = nc.dram_tensor("linbr", [128, 128], F32, kind="ExternalInput")
    identb_in = nc.dram_tensor("identb", [128, 128], BF16, kind="ExternalInput")
    identf_in = nc.dram_tensor("identf", [128, 128], F32, kind="ExternalInput")
    ones_in = nc.dram_tensor("ones", [128, 128], F32, kind="ExternalInput")
    padm_in = nc.dram_tensor("padm", [128, 1], F32, kind="ExternalInput")
    y_out = nc.dram_tensor("y", [NSHP, 128], F32, kind="ExternalOutput")

    ag_in = nc.dram_tensor("ag_in", [NSHA, 128], BF16)
    table2 = nc.dram_tensor("table2", [TBL, 128], BF16, addr_space="Shared")
    table = nc.dram_tensor("table", [TBL, 128], BF16)
    st_in = nc.dram_tensor("st_in", [128, 2], F32)
    st_out = nc.dram_tensor("st_out", [NC * 128, 2], F32, addr_space="Shared")

    RG = [list(range(NC))]

    with tile.TileContext(nc) as tc, ExitStack() as ctx:
        # ---- persistent SBUF (bufs=1 pools)
        P = ctx.enter_context(tc.tile_pool(name="persist", bufs=1))
        rtab = P.tile([128, NSHP], BF16, tag="rtab")      # own x~_l rows
        outrows = P.tile([128, NSHP], BF16, tag="outrows")  # pre-norm out rows
        aggT = P.tile([128, NSHP], F32R, tag="aggT")      # agg^T / out^T slab
        outfin = P.tile([128, NSHP], F32, tag="outfin")
        idx_sb = P.tile([128, 8 * rounds_tot], I16, tag="idx")
        mask_sb = P.tile([128, 2 * rounds_tot], BF16, tag="mask")
        W_sb = P.tile([128, n_layers * 128], F32R, tag="W")
        asl_t = P.tile([128, GRP * 256], BF16, tag="asl_t")
        adl_t = P.tile([128, GRP * 128], BF16, tag="adl_t")
        biasc_sb = P.tile([128, n_layers], F32, tag="biasc")
        linw_sb = P.tile([128, n_layers * 128], F32R, tag="linw")
        linbr_sb = P.tile([128, 128], F32, tag="linbr")
        identb = P.tile([128, 128], BF16, tag="identb")
        identf = P.tile([128, 128], F32, tag="identf")
        ones_sb = P.tile([128, 128], F32, tag="ones")
        padm_sb = P.tile([128, 1], F32, tag="padm")
        as_own = P.tile([128, TILES], F32, tag="as_own")
        ad_own = P.tile([128, TILES], F32, tag="ad_own")
        wself = P.tile([128, TILES], F32, tag="wself")
        stp = P.tile([128, 2], F32, tag="stp")
        statp = P.tile([128, 4], BF16, tag="statp")
        strow = P.tile([4, 128], BF16, tag="strow")
        st32 = P.tile([32, 128], BF16, tag="st32")
        stf = P.tile([32, 128], F32, tag="stf")
        gst = P.tile([128, 32], F32, tag="gst")
        st8 = P.tile([128, 16], F32, tag="st8")
        gstat = P.tile([128, 2], F32, tag="gstat")
        nmslab = P.tile([128, NORM_CHUNK], F32, tag="nmslab")
        gtmp = P.tile([128, 16], F32, tag="gtmp")

        # ---- input loads
        nc.sync.dma_start(idx_sb[:], idx_in.ap())
        nc.sync.dma_start(mask_sb[:], mask_in.ap())
        nc.sync.dma_start(W_sb[:].rearrange("a (l b) -> a l b", b=128), W_in.ap().rearrange("l a b -> a l b"))
        nc.sync.dma_start(biasc_sb[:], biasc_in.ap())
        nc.sync.dma_start(linw_sb[:].rearrange("a (l b) -> a l b", b=128), linw_in.ap().rearrange("l a b -> a l b"))
        nc.sync.dma_start(linbr_sb[:], linbr_in.ap())
        nc.sync.dma_start(identb[:], identb_in.ap())
        nc.sync.dma_start(identf[:], identf_in.ap())
        nc.sync.dma_start(ones_sb[:], ones_in.ap())
        nc.sync.dma_start(padm_sb[:], padm_in.ap())
        nc.sync.dma_start(rtab[:].rearrange("p (t f) -> p t f", f=128),
                          xown_in.ap().rearrange("(t p) f -> p t f", p=128))

        # ---- working pools
        PD = ctx.enter_context(tc.tile_pool(name="pd", bufs=2, space="PSUM"))
        PT = ctx.enter_context(tc.tile_pool(name="pt", bufs=2, space="PSUM"))
        PA = ctx.enter_context(tc.tile_pool(name="pa", bufs=2, space="PSUM"))
        PM = ctx.enter_context(tc.tile_pool(name="pm", bufs=2, space="PSUM"))
        GSEG = ctx.enter_context(tc.tile_pool(name="gseg", bufs=2))
        GN = ctx.enter_context(tc.tile_pool(name="gn", bufs=2))
        JNK8 = ctx.enter_context(tc.tile_pool(name="jnk8", bufs=2))
        GW8 = ctx.enter_context(tc.tile_pool(name="gw8", bufs=2))
        SC = ctx.enter_context(tc.tile_pool(name="sc", bufs=3))
        SS = ctx.enter_context(tc.tile_pool(name="ss", bufs=4))
        RP = ctx.enter_context(tc.tile_pool(name="rp", bufs=3))

        # mask col offset per tile (processing order)
        mcol = {}
        col = 0
        for t in range(TILES):
            mcol[t] = col
            col += int(D[t])
        # idx col offsets (wrapped layout: 8 cols per round)
        idx_off = {}
        ic = 0
        for si, seg in enumerate(segs):
            seg_r = int(sum(D[t] for t in seg))
            idx_off[si] = (ic, seg_r)
            ic += 8 * seg_r

        for l in range(n_layers):
            src_tab = xrows_in if l == 0 else table
            wsl = W_sb[:, l * 128:(l + 1) * 128]
            lwl = linw_sb[:, l * 128:(l + 1) * 128]

            # ---- per-layer tiled attention vectors
            nc.sync.dma_start(asl_t[:], asrc_in.ap()[l])
            nc.sync.dma_start(adl_t[:], adst_in.ap()[l])

            # ---- own alpha dots (grouped: TT mult + 3D reduce) + self weight
            for t0 in range(0, TILES, GRP):
                g = min(GRP, TILES - t0)
                gsl = slice(t0 * 128, (t0 + g) * 128)
                for avec, aout in ((asl_t, as_own), (adl_t, ad_own)):
                    pr = JNK8.tile([128, GRP * 128], BF16, tag="jnk8")
                    nc.vector.tensor_tensor(pr[:, :g * 128], rtab[:, gsl],
                                            avec[:, :g * 128], op=OP.mult)
                    nc.vector.reduce_sum(
                        out=aout[:, t0:t0 + g],
                        in_=pr[:, :g * 128].rearrange("p (r e) -> p r e", e=128),
                        axis=AX.X)
            zs = SC.tile([128, TILES], F32, tag="zself")
            nc.vector.tensor_tensor(zs[:], as_own[:], ad_own[:], op=OP.add)
            zs2 = SC.tile([128, TILES], F32, tag="zself")
            nc.vector.scalar_tensor_tensor(
                out=zs2[:], in0=zs[:], scalar=NEG, in1=zs[:], op0=OP.mult, op1=OP.max)
            nc.scalar.activation(wself[:], zs2[:], AF.Exp)

            # ---- aggregation over segments (pair-packed 512B gathers)
            src_pairs = src_tab.ap().rearrange("(q two) f -> q (two f)", two=2)
            for si, seg in enumerate(segs):
                seg_r = int(sum(D[t] for t in seg))
                gbuf = GSEG.tile([128, SEG_MAX_ROUNDS * 256], BF16, tag="gseg")
                g3 = gbuf[:].rearrange("p (r e) -> p r e", e=256)
                ic0, _sr = idx_off[si]
                if variant == "nogather":
                    nc.vector.memset(gbuf[:, :seg_r * 256], 0.0)
                else:
                    sh = seg_r // 2
                    if sh:
                        nc.gpsimd.dma_gather(
                            g3[:, :sh, :], src_pairs,
                            idx_sb[:, ic0:ic0 + 8 * sh],
                            128 * sh, 128 * sh, 256,
                            single_packet=SINGLE_PACKET, queue_num=0)
                    nc.gpsimd.dma_gather(
                        g3[:, sh:seg_r, :], src_pairs,
                        idx_sb[:, ic0 + 8 * sh:ic0 + 8 * seg_r],
                        128 * (seg_r - sh), 128 * (seg_r - sh), 256,
                        single_packet=SINGLE_PACKET,
                        queue_num=1 if sh else 0)

                # per-tile processing (2 interleaved halves per round)
                cum = 0
                for t in seg:
                    Dt = int(D[t])
                    tsl = slice(t * 128, (t + 1) * 128)
                    c0 = cum
                    asg = SC.tile([128, 2 * DMX], F32, tag="asg")
                    for q0 in range(0, Dt, GRP):
                        q = min(GRP, Dt - q0)
                        pr = JNK8.tile([128, GRP * 256], BF16, tag="jnk8")
                        nc.vector.tensor_tensor(
                            pr[:, :q * 256],
                            gbuf[:, (c0 + q0) * 256:(c0 + q0 + q) * 256],
                            asl_t[:, :q * 256], op=OP.mult)
                        nc.vector.reduce_sum(
                            out=asg[:, 2 * q0:2 * (q0 + q)],
                            in_=pr[:, :q * 256].rearrange(
                                "p (r e) -> p r e", e=128),
                            axis=AX.X)
                    zt = SC.tile([128, 2 * DMX], F32, tag="zt")
                    nc.vector.tensor_scalar_add(zt[:, :2 * Dt], asg[:, :2 * Dt],
                                                ad_own[:, t:t + 1])
                    zl = SC.tile([128, 2 * DMX], F32, tag="zl")
                    nc.vector.scalar_tensor_tensor(
                        out=zl[:, :2 * Dt], in0=zt[:, :2 * Dt], scalar=NEG,
                        in1=zt[:, :2 * Dt], op0=OP.mult, op1=OP.max)
                    ew = SC.tile([128, 2 * DMX], F32, tag="ew")
                    nc.scalar.activation(ew[:, :2 * Dt], zl[:, :2 * Dt], AF.Exp)
                    ewm = SC.tile([128, 2 * DMX], F32, tag="ewm")
                    S = SS.tile([128, 1], F32, tag="S")
                    nc.vector.scalar_tensor_tensor(
                        out=ewm[:, :2 * Dt], in0=ew[:, :2 * Dt], scalar=1.0,
                        in1=mask_sb[:, 2 * mcol[t]:2 * (mcol[t] + Dt)],
                        op0=OP.mult, op1=OP.mult, accum_out=S[:])
                    Sp = SS.tile([128, 1], F32, tag="Sp")
                    nc.vector.scalar_tensor_tensor(
                        out=Sp[:], in0=S[:], scalar=SEPS, in1=wself[:, t:t + 1],
                        op0=OP.add, op1=OP.add)
                    rec = SS.tile([128, 1], F32, tag="rec")
                    nc.vector.reciprocal(rec[:], Sp[:])

                    pa = PA.tile([128, 128], F32, tag="pa")
                    pav = pa[:]
                    nmm = -(-(2 * Dt) // 4)
                    mi = 0
                    use_dve = (t % 2 == 0)
                    for q0 in range(0, Dt, GRP):
                        q = min(GRP, Dt - q0)
                        gw = GW8.tile([128, GRP * 256], BF16, tag="gw8")
                        if use_dve:
                            ew_ap = ewm[:, 2 * q0:2 * (q0 + q)]
                            e3 = bass.AP(
                                ew_ap.tensor, ew_ap.offset,
                                [list(ew_ap.ap[0]),
                                 [ew_ap.ap[1][0], 2 * q], [0, 128]])
                            nc.vector.tensor_tensor(
                                gw[:, :q * 256].rearrange(
                                    "p (r e) -> p r e", e=128),
                                gbuf[:, (c0 + q0) * 256:(c0 + q0 + q) * 256]
                                .rearrange("p (r e) -> p r e", e=128),
                                e3, op=OP.mult)
                        else:
                            for kk in range(2 * q):
                                nc.scalar.activation(
                                    gw[:, kk * 128:(kk + 1) * 128],
                                    gbuf[:, (c0 + q0) * 256 + kk * 128:
                                         (c0 + q0) * 256 + (kk + 1) * 128],
                                    AF.Copy,
                                    scale=ewm[:, 2 * q0 + kk:2 * q0 + kk + 1])
                        for m0 in range(0, 2 * q, 4):
                            m = min(4, 2 * q - m0)
                            pz = bass.AP(
                                pav.tensor, pav.offset,
                                [list(pav.ap[0]), [0, m], list(pav.ap[1])])
                            nc.tensor.matmul(
                                pz, identb[:],
                                gw[:, m0 * 128:(m0 + m) * 128],
                                start=(mi == 0), stop=(mi == nmm - 1))
                            mi += 1

                    acc1 = RP.tile([128, 128], F32, tag="acc1")
                    nc.vector.scalar_tensor_tensor(
                        out=acc1[:], in0=rtab[:, tsl], scalar=wself[:, t:t + 1],
                        in1=pa[:], op0=OP.mult, op1=OP.add)
                    row = RP.tile([128, 128], F32, tag="row")
                    nc.scalar.activation(row[:], acc1[:], AF.Copy, scale=rec[:])
                    if t == TILES - 1:
                        nc.vector.tensor_scalar_mul(row[:], row[:], padm_sb[:])
                    # transpose agg rows into aggT slab
                    ptf = PM.tile([128, 128], F32, tag="pm")
                    nc.tensor.transpose(ptf[:], row[:], identf[:])
                    nc.vector.tensor_copy(aggT[:, tsl], ptf[:])
                    cum += Dt

            # ---- dense post-aggregation: out^T = W^T @ agg^T + bias
            for ch0 in range(0, NSHP, 512):
                chsz = min(512, NSHP - ch0)
                pd = PD.tile([128, 512], F32, tag="pd")
                nc.tensor.matmul(pd[:, :chsz], wsl, aggT[:, ch0:ch0 + chsz],
                                 start=True, stop=True)
                nc.vector.tensor_scalar_add(aggT[:, ch0:ch0 + chsz], pd[:, :chsz],
                                            biasc_sb[:, l:l + 1])

            # ---- pairnorm stats partials on out^T (feat-major)
            fsum = SS.tile([128, 1], F32, tag="fsum")
            nc.vector.reduce_sum(out=fsum[:], in_=aggT[:], axis=AX.X)
            sqc = SS.tile([128, 1], F32, tag="sqc")
            nc.scalar.activation(outrows[:], aggT[:], AF.Square, accum_out=sqc[:])
            nc.vector.tensor_copy(stp[:, :1], fsum[:])
            nc.vector.tensor_copy(stp[:, 1:2], sqc[:])

            if l < n_layers - 1:
                # ---- payload: transpose out^T -> out rows (bf16)
                for t in range(TILES):
                    tsl = slice(t * 128, (t + 1) * 128)
                    ptf = PM.tile([128, 128], F32, tag="pm")
                    nc.tensor.transpose(ptf[:], aggT[:, tsl].bitcast(F32),
                                        identf[:])
                    nc.scalar.activation(outrows[:, tsl], ptf[:], AF.Copy)
                # stats hi/lo pack into statp [128,4] = (fh, qh, fl, ql) cols
                nc.vector.tensor_copy(statp[:, 0:2], stp[:])
                hif = SS.tile([128, 2], F32, tag="hif")
                nc.vector.tensor_copy(hif[:], statp[:, 0:2])
                lof = SS.tile([128, 2], F32, tag="lof")
                nc.vector.tensor_tensor(lof[:], stp[:], hif[:], op=OP.subtract)
                nc.vector.tensor_copy(statp[:, 2:4], lof[:])
                pst = PT.tile([128, 128], BF16, tag="pt")
                nc.tensor.transpose(pst[:4, :], statp[:], identb[:])
                nc.vector.tensor_copy(strow[:], pst[:4, :])
                # DMA payload
                nc.sync.dma_start(ag_in.ap()[:NSHP].rearrange("(t p) f -> p t f", p=128),
                                  outrows[:].rearrange("p (t f) -> p t f", f=128))
                nc.sync.dma_start(ag_in.ap()[NSHP:NSHP + 4], strow[:])
                if do_coll:
                    nc.gpsimd.collective_compute(
                        "AllGather", OP.bypass, replica_groups=RG,
                        ins=[ag_in.ap()], outs=[table2.ap()])
                else:
                    nc.sync.dma_start(table2.ap()[:NSHA], ag_in.ap())

                # ---- global stats from the 8 cores' tail rows
                for cc in range(NC):
                    nc.sync.dma_start(
                        st32[cc * 4:(cc + 1) * 4, :],
                        table2.ap()[cc * NSHA + NSHP:cc * NSHA + NSHP + 4, :])
                nc.vector.tensor_copy(stf[:], st32[:])
                pg = PM.tile([128, 128], F32, tag="pm")
                nc.tensor.transpose(pg[:, :32], stf[:], identf[:32, :32])
                nc.vector.tensor_copy(gst[:], pg[:, :32])
                g4 = gst[:].rearrange("p (c a) -> p c a", a=4)
                nc.vector.tensor_tensor(
                    st8[:].rearrange("p (c s) -> p c s", s=2),
                    g4[:, :, 0:2], g4[:, :, 2:4], op=OP.add)
                nc.vector.reduce_sum(
                    out=gstat[:],
                    in_=st8[:].rearrange("p (c s) -> p s c", s=2),
                    axis=AX.X)
            else:
                # ---- last layer: lone stats AllGather
                nc.sync.dma_start(st_in.ap(), stp[:])
                if do_coll:
                    nc.gpsimd.collective_compute(
                        "AllGather", OP.bypass, replica_groups=RG,
                        ins=[st_in.ap()], outs=[st_out.ap()])
                else:
                    for rr in range(NC):
                        nc.sync.dma_start(st_out.ap()[rr * 128:(rr + 1) * 128],
                                          st_in.ap())
                nc.sync.dma_start(gtmp[:].rearrange("p (r c) -> p r c", c=2),
                                  st_out.ap().rearrange("(r p) c -> p r c", p=128))
                nc.vector.reduce_sum(out=gstat[:],
                                     in_=gtmp[:].rearrange("p (r c) -> p c r", c=2),
                                     axis=AX.X)

            # ---- mu / denom / scale (gstat = [fsum_g, sqsum_g] per feature)
            mu = SS.tile([128, 1], F32, tag="mu")
            nc.vector.tensor_scalar_mul(mu[:], gstat[:, :1], 1.0 / N)
            st2 = SS.tile([128, 2], F32, tag="st2")
            nc.vector.tensor_copy(st2[:, :1], gstat[:, 1:2])
            nc.vector.tensor_tensor(st2[:, 1:2], mu[:], mu[:], op=OP.mult)
            p2 = PM.tile([128, 128], F32, tag="pm")
            nc.tensor.matmul(p2[:1, :2], ones_sb[:, :1], st2[:],
                             start=True, stop=True)
            tot = SS.tile([1, 2], F32, tag="tot")
            nc.vector.tensor_copy(tot[:], p2[:1, :2])
            v3 = SS.tile([1, 1], F32, tag="v3")
            nc.vector.tensor_scalar(v3[:], tot[:, :1], 1.0 / N, PEPS,
                                    op0=OP.mult, op1=OP.add)
            v4 = SS.tile([1, 1], F32, tag="v4")
            nc.vector.tensor_tensor(v4[:], v3[:], tot[:, 1:2], op=OP.subtract)
            den = SS.tile([1, 1], F32, tag="den")
            nc.scalar.activation(den[:], v4[:], AF.Sqrt)
            invd = SS.tile([1, 1], F32, tag="invd")
            nc.vector.reciprocal(invd[:], den[:])
            pb1 = PM.tile([128, 128], F32, tag="pm")
            nc.tensor.matmul(pb1[:, :1], ones_sb[:1, :], invd[:],
                             start=True, stop=True)
            invdr = SS.tile([128, 1], F32, tag="invdr")
            nc.vector.tensor_copy(invdr[:], pb1[:, :1])
            nms = SS.tile([128, 1], F32, tag="nms")
            nc.vector.tensor_scalar(nms[:], mu[:], invdr[:], -1.0,
                                    op0=OP.mult, op1=OP.mult)

            # ---- own feat-major norm+gelu (in place on aggT) + JK increment
            for ch0 in range(0, NSHP, 512):
                chsz = min(512, NSHP - ch0)
                csl = slice(ch0, ch0 + chsz)
                nc.scalar.activation(aggT[:, csl], aggT[:, csl], GELU,
                                     bias=nms[:], scale=invdr[:])
            for t in range(TILES):
                tsl = slice(t * 128, (t + 1) * 128)
                pf = PM.tile([128, 128], F32, tag="pm")
                nc.tensor.matmul(pf[:], aggT[:, tsl], lwl, start=True, stop=True)
                if l == 0:
                    nc.vector.scalar_tensor_tensor(
                        out=outfin[:, tsl], in0=pf[:], scalar=1.0, in1=linbr_sb[:],
                        op0=OP.mult, op1=OP.add)
                else:
                    nc.vector.scalar_tensor_tensor(
                        out=outfin[:, tsl], in0=pf[:], scalar=1.0,
                        in1=outfin[:, tsl], op0=OP.mult, op1=OP.add)

            if l < n_layers - 1:
                # ---- nms replicated-row slab for row-space norm
                pnr = PM.tile([128, 128], F32, tag="pm")
                nc.tensor.transpose(pnr[:1, :], nms[:], identf[:])
                nmsrow = SS.tile([1, 128], F32, tag="nmsrow")
                nc.vector.tensor_copy(nmsrow[:], pnr[:1, :])
                prep = PM.tile([128, 128], F32, tag="pm")
                nc.tensor.matmul(prep[:], ones_sb[:1, :], nmsrow[:],
                                 start=True, stop=True)
                nrep = SS.tile([128, 128], F32, tag="nrep")
                nc.vector.tensor_copy(nrep[:], prep[:])
                for k in range(NORM_CHUNK // 128):
                    nc.vector.tensor_copy(nmslab[:, k * 128:(k + 1) * 128], nrep[:])

                # ---- own rows norm+gelu -> rtab (next layer's own rows)
                for h0, hw in ((0, 3072), (3072, NSHP - 3072)):
                    hs = slice(h0, h0 + hw)
                    nc.vector.scalar_tensor_tensor(
                        out=outrows[:, hs], in0=outrows[:, hs], scalar=invdr[:],
                        in1=nmslab[:, :hw], op0=OP.mult, op1=OP.add)
                    nc.scalar.activation(rtab[:, hs], outrows[:, hs], GELU)

                # ---- full-table norm+gelu: table2 -> table (chunked)
                nch = TBL // NORM_CHUNK
                for ci in range(nch):
                    r0 = ci * NORM_CHUNK
                    cin = GN.tile([128, NORM_CHUNK], BF16, tag="gnb")
                    nc.sync.dma_start(
                        cin[:].rearrange("p (t f) -> p t f", f=128),
                        table2.ap()[r0:r0 + NORM_CHUNK].rearrange(
                            "(t p) f -> p t f", p=128))
                    nc.vector.scalar_tensor_tensor(
                        out=cin[:], in0=cin[:], scalar=invdr[:],
                        in1=nmslab[:], op0=OP.mult, op1=OP.add)
                    nc.scalar.activation(cin[:], cin[:], GELU)
                    nc.sync.dma_start(
                        table.ap()[r0:r0 + NORM_CHUNK].rearrange(
                            "(t p) f -> p t f", p=128),
                        cin[:].rearrange("p (t f) -> p t f", f=128))

        # ---- write final output
        nc.sync.dma_start(y_out.ap().rearrange("(t p) f -> p t f", p=128),
                          outfin[:].rearrange("p (t f) -> p t f", f=128))

    return nc


def make_inputs(inputs, meta, percore, n_layers=L):
    """Build per-core in_maps from the full problem inputs."""
    x = np.asarray(inputs["x"], np.float32)
    W0 = np.asarray(inputs["W0"], np.float32)
    Ws = np.asarray(inputs["Ws"], np.float32)
    att_src = np.asarray(inputs["att_src"], np.float32)
    att_dst = np.asarray(inputs["att_dst"], np.float32)
    bias = np.asarray(inputs["bias"], np.float32)
    lin_w = np.asarray(inputs["lin_w"], np.float32)
    lin_b = np.asarray(inputs["lin_b"], np.float32)

    Wl = [W0] + [Ws[i] for i in range(n_layers - 1)]
    Wst = np.stack(Wl).astype(np.float32)
    asrc = np.stack([np.tile(Wl[i] @ att_src[i], (128, 2 * GRP))
                     for i in range(n_layers)]).astype(ml_dtypes.bfloat16)
    adst = np.stack([np.tile(Wl[i] @ att_dst[i], (128, GRP))
                     for i in range(n_layers)]).astype(ml_dtypes.bfloat16)
    biasc = np.stack([bias[i] for i in range(n_layers)], axis=1).astype(np.float32)
    linw = np.stack([lin_w[i * HID:(i + 1) * HID] for i in range(n_layers)]).astype(np.float32)
    linbr = np.tile(lin_b, (128, 1)).astype(np.float32)
    identb = np.eye(128, dtype=ml_dtypes.bfloat16)
    identf = np.eye(128, dtype=np.float32)
    ones = np.ones((128, 128), np.float32)
    padm = np.zeros((128, 1), np.float32)
    padm[:NSH - (TILES - 1) * 128] = 1.0

    # full layer-0 gather table: x rows in permuted layout, replicated
    xrows = np.zeros((TBL, 128), ml_dtypes.bfloat16)
    xb = x.astype(ml_dtypes.bfloat16)
    in_maps = []
    xowns = []
    for c in range(NC):
        order = percore[c]["order"]
        xs = xb[c * NSH:(c + 1) * NSH][order]          # [6250,128] sorted
        xrows[c * NSHA:c * NSHA + NSH] = xs
        xo = np.zeros((NSHP, 128), ml_dtypes.bfloat16)
        xo[:NSH] = xs
        xowns.append(xo)
    for c in range(NC):
        in_maps.append({
            "xrows": xrows, "xown": xowns[c],
            "idx": percore[c]["idx"],
            "mask": percore[c]["mask"].astype(ml_dtypes.bfloat16),
            "Wst": Wst, "asrc": asrc, "adst": adst, "biasc": biasc,
            "linw": linw, "linbr": linbr, "identb": identb, "identf": identf,
            "ones": ones, "padm": padm,
        })
    return in_maps


def assemble_output(results, percore):
    """Concatenate per-core outputs, undoing the degree-sort permutation."""
    out = np.empty((N, HID), np.float32)
    for c in range(NC):
        order = percore[c]["order"]
        yc = results[c]["y"][:NSH]     # sorted order
        out[c * NSH + order] = yc
    return out


# ---------------------------------------------------------------------------
# kernel() entry point
# ---------------------------------------------------------------------------
_CACHE = {}


def _get_compiled(edge_key, edge_index):
    if edge_key not in _CACHE:
        meta, percore = preprocess(edge_index)
        nc = bacc.Bacc("TRN2", target_bir_lowering=False, debug=False,
                       num_devices=NC, num_swdge_queues=N_SWDGE_Q)
        build(nc, meta, n_layers=L, sim_safe=False)
        nc.compile()
        _CACHE[edge_key] = (nc, meta, percore)
    return _CACHE[edge_key]


def kernel(**inputs):
    from concourse.bass_utils import run_bass_kernel_spmd
    edge_index = np.asarray(inputs["edge_index"])
    edge_key = hash(edge_index.tobytes())
    nc, meta, percore = _get_compiled(edge_key, edge_index)
    in_maps = make_inputs(inputs, meta, percore, n_layers=L)
    res = run_bass_kernel_spmd(nc, in_maps, list(range(NC)))
    return assemble_output(res.results, percore)
